# revision 13
# baseline (speedup 1.0000x reference)
"""Causal self-attention (B=4, T=4096, D=768, single head, fp32) on 8 TRN2
NeuronCores.

Sharding: core <-> (batch b = core//2, parity h = core%2). Each core handles
the 16 query tiles (128 rows) at global tile index g = 2i + h for local
i = 0..15 (parity interleave balances causal work across the pair to ~3%).
Per local q-tile i the kernel computes scores against keys [0, 256*(i+1)):
columns below 256*i are always causally allowed for both parities; the last
256 columns are fixed up with a per-core input mask tile ([tri | -inf] for
h=0, [0 | tri] for h=1). This keeps the SPMD program identical on all cores
while wasting only ~6% of the causal-skipped flops.

Compute path (per core):
  - Q^T projected once to SBUF (lhsT = W_q^T tiles, rhs = streamed x^T).
  - kv processed in 8 superblocks of 512: K^T/V projected just-in-time from
    streamed x^T, consumed from SBUF.
  - S = Q.K^T via fp32r (FP22) matmuls; P = exp(S/sqrt(D)) on ACT with
    per-row sums (no max pass needed: scores are ~N(0,1), exp can't
    overflow); P transposed 128x128 on the PE; O accumulated in PSUM per
    superblock then merged into an SBUF accumulator; q-tiles normalize by
    1/l and DMA out as soon as their key range completes.
"""

import os
import sys
from contextlib import ExitStack

import numpy as np

if "/opt/trn_rl_repo" not in sys.path:
    sys.path.insert(0, "/opt/trn_rl_repo")

B, T, D = 4, 4096, 768
N_CORES = 8
QTILES = 16          # local q-tiles per core, 128 rows each
EC = D // 128        # 6 e/d chunks of 128
SB = 8               # kv superblocks
SBW = 512            # superblock width (keys)
NEG = -1.0e9
SCALE = 1.0 / float(np.sqrt(D))

_CACHE = {}


def _patch_tile_drain():
    """This walrus build accepts only one sync wait per CTRL instruction;
    TileContext's tail drain carries one wait per outstanding proc. Split
    them onto individual SP no-ops (SP executes sequentially, so semantics
    are unchanged)."""
    import concourse.mybir as mybir
    import concourse.tile as tile
    from concourse.vector_clock import ScopedClock

    if getattr(tile.TileContext, "_drain_split_patch", False):
        return

    def _split_drain_and_barrier(self, tick_clock, wait_clock):
        nc = self.nc
        carrier = nc.sync.nop(nofuse=True)
        wait_clock.add_sem_waits(
            carrier.ins, ScopedClock({None: tick_clock.global_clock})
        )
        si = carrier.ins.sync_info
        waits = list(si.on_wait) if si is not None else []
        carrier.ins.sync_info = mybir.SyncInfo(on_wait=waits[:1], on_update=[])
        for w in waits[1:]:
            n = nc.sync.nop(nofuse=True)
            n.ins.sync_info = mybir.SyncInfo(on_wait=[w], on_update=[])
        nc.sync.drain()
        nc.all_engine_barrier()
        assert self.sems is not None
        popped = nc._tile_sem_poison_stack.pop()
        assert popped is self._sem_poison
        nc.clear_and_free_semaphores(list(self.sems.allocated().values()))
        nc.all_engine_barrier()

    tile.TileContext._drain_and_barrier = _split_drain_and_barrier
    tile.TileContext._drain_split_patch = True


def _hoist_multi_waits(nc):
    """This walrus build encodes at most ONE sync wait per instruction
    descriptor (setupSyncWait raises 'Too many sync wait commands' otherwise).
    Tile's sem assignment can put several waits on one instruction; hoist the
    extras onto same-engine no-ops inserted immediately before it — the
    engine executes them sequentially, so the wait semantics are unchanged."""
    import concourse.mybir as mybir

    n = 0
    for fn in nc.m.functions:
        for bb in fn.blocks:
            insts = bb.instructions
            out = []
            for ins in insts:
                si = ins.sync_info
                waits = list(si.on_wait) if si is not None else []
                if len(waits) > 1:
                    for w in waits[:-1]:
                        nop = mybir.InstNoOp(
                            name=f"I-hoistw-{nc.next_id()}",
                            engine=ins.engine,
                            ins=[],
                            outs=[],
                            sync_info=mybir.SyncInfo(on_wait=[w], on_update=[]),
                        )
                        out.append(nop)
                        n += 1
                    ins.sync_info = mybir.SyncInfo(
                        on_wait=[waits[-1]], on_update=list(si.on_update)
                    )
                out.append(ins)
            insts[:] = out
    return n


def _build_program(hoist=True):
    import concourse.bass as bass
    import concourse.mybir as mybir
    import concourse.tile as tile

    _patch_tile_drain()
    f32 = mybir.dt.float32
    f32r = mybir.dt.float32r
    Exp = mybir.ActivationFunctionType.Exp
    Copy = mybir.ActivationFunctionType.Copy
    Add = mybir.AluOpType.add
    AXX = mybir.AxisListType.X

    nc = bass.Bass()
    xkvT = nc.dram_tensor("xkvT", [128, EC, T], f32r, kind="ExternalInput")
    xqT = nc.dram_tensor("xqT", [128, EC, QTILES * 128], f32r, kind="ExternalInput")
    wqT = nc.dram_tensor("wqT", [128, EC, D], f32r, kind="ExternalInput")
    wkT = nc.dram_tensor("wkT", [128, EC, D], f32r, kind="ExternalInput")
    wvT = nc.dram_tensor("wvT", [128, EC, D], f32r, kind="ExternalInput")
    maskab = nc.dram_tensor("maskab", [128, 256], f32, kind="ExternalInput")
    ident = nc.dram_tensor("ident", [128, 128], f32r, kind="ExternalInput")
    out_d = nc.dram_tensor("out", [QTILES * 128, D], f32, kind="ExternalOutput")

    with tile.TileContext(nc) as tc:
        with (
            tc.tile_pool(name="consts", bufs=1) as cpool,
            tc.tile_pool(name="wk", bufs=1) as wkpool,
            tc.tile_pool(name="qt", bufs=1) as qtpool,
            tc.tile_pool(name="oacc", bufs=1) as oapool,
            tc.tile_pool(name="ps_s", bufs=2, space="PSUM") as ps_s,
            tc.tile_pool(name="ps_t", bufs=2, space="PSUM") as ps_t,
            tc.tile_pool(name="ps_o", bufs=2, space="PSUM") as ps_o,
        ):
            ident_t = cpool.tile([128, 128], f32r, tag="ident")
            nc.sync.dma_start(out=ident_t[:], in_=ident[:])
            mask_t = cpool.tile([128, 256], f32, tag="mask")
            nc.sync.dma_start(out=mask_t[:], in_=maskab[:])
            wk_t = wkpool.tile([128, EC, D], f32r, tag="wk")
            nc.sync.dma_start(out=wk_t[:], in_=wkT[:])
            wv_t = wkpool.tile([128, EC, D], f32r, tag="wv")
            nc.sync.dma_start(out=wv_t[:], in_=wvT[:])

            # ---- Q projection: QT[e_part, e_chunk, q] resident in SBUF ----
            # W_q / x_q tiles live in a scoped pool freed before the kv loop.
            qt_t = qtpool.tile([128, EC, QTILES * 128], f32r, tag="qt")
            with tc.tile_pool(name="qproj", bufs=2) as qppool:
                wq_t = qppool.tile([128, EC, D], f32r, tag="wq", bufs=1)
                nc.sync.dma_start(out=wq_t[:], in_=wqT[:])
                for qc in range(4):  # 4 chunks of 512 query columns
                    xq_t = qppool.tile([128, EC, 512], f32r, tag="xq")
                    nc.sync.dma_start(
                        out=xq_t[:], in_=xqT[:, :, qc * 512 : (qc + 1) * 512]
                    )
                    for m in range(EC):
                        ps = ps_s.tile([128, 512], f32, tag="s")
                        for j in range(EC):
                            nc.tensor.matmul(
                                ps[:],
                                wq_t[:, j, m * 128 : (m + 1) * 128],
                                xq_t[:, j, :],
                                start=(j == 0),
                                stop=(j == EC - 1),
                            )
                        nc.scalar.copy(
                            out=qt_t[:, m, qc * 512 : (qc + 1) * 512], in_=ps[:]
                        )

            oacc_t = oapool.tile([128, QTILES * D], f32, tag="oacc")
            lparts_t = oapool.tile([128, QTILES * SB], f32, tag="lparts")

            # ---- kv superblocks ----
            attn_pools = ExitStack()
            xspool = attn_pools.enter_context(tc.tile_pool(name="xs", bufs=2))
            kvpool = attn_pools.enter_context(tc.tile_pool(name="kv", bufs=1))
            ppool = attn_pools.enter_context(tc.tile_pool(name="p", bufs=3))
            ptpool = attn_pools.enter_context(tc.tile_pool(name="pt", bufs=3))
            spool = attn_pools.enter_context(tc.tile_pool(name="small", bufs=4))
            obpool = attn_pools.enter_context(tc.tile_pool(name="ob", bufs=2))
            for sb in range(SB):
                xkv_t = xspool.tile([128, EC, SBW], f32r, tag="xs")
                nc.sync.dma_start(
                    out=xkv_t[:], in_=xkvT[:, :, sb * SBW : (sb + 1) * SBW]
                )
                # K^T for this superblock: [e_part, e_chunk, kv]
                kt_t = kvpool.tile([128, EC, SBW], f32r, tag="kt")
                for m in range(EC):
                    ps = ps_s.tile([128, SBW], f32, tag="s")
                    for j in range(EC):
                        nc.tensor.matmul(
                            ps[:],
                            wk_t[:, j, m * 128 : (m + 1) * 128],
                            xkv_t[:, j, :],
                            start=(j == 0),
                            stop=(j == EC - 1),
                        )
                    nc.scalar.copy(out=kt_t[:, m, :], in_=ps[:])
                # V for this superblock: [kv_part, kv_tile, e]
                v_t = kvpool.tile([128, SBW // 128, D], f32r, tag="v")
                for kt in range(SBW // 128):
                    ps = ps_o.tile([128, D], f32, tag="o")
                    for j in range(EC):
                        for lo, n in ((0, 512), (512, 256)):  # psum-bank aligned
                            nc.tensor.matmul(
                                ps[:, lo : lo + n],
                                xkv_t[:, j, kt * 128 : (kt + 1) * 128],
                                wv_t[:, j, lo : lo + n],
                                start=(j == 0),
                                stop=(j == EC - 1),
                            )
                    nc.scalar.copy(out=v_t[:, kt, :], in_=ps[:])

                # ---- attention for q-tiles active in this superblock ----
                for i in range(2 * sb, QTILES):
                    w = 256 if i == 2 * sb else SBW
                    nkt = w // 128
                    terminal = i in (2 * sb, 2 * sb + 1)

                    ps_sx = ps_s.tile([128, SBW], f32, tag="s")
                    for j in range(EC):
                        nc.tensor.matmul(
                            ps_sx[:, :w],
                            qt_t[:, j, i * 128 : (i + 1) * 128],
                            kt_t[:, j, :w],
                            start=(j == 0),
                            stop=(j == EC - 1),
                        )
                    if terminal:
                        nc.vector.tensor_add(
                            ps_sx[:, w - 256 : w], ps_sx[:, w - 256 : w], mask_t[:]
                        )
                    p_t = ppool.tile([128, SBW], f32r, tag="p")
                    nc.scalar.activation(
                        p_t[:, :w],
                        ps_sx[:, :w],
                        Exp,
                        scale=SCALE,
                        accum_out=lparts_t[:, i * SB + sb : i * SB + sb + 1],
                    )
                    po = ps_o.tile([128, D], f32, tag="o")
                    for kt in range(nkt):
                        pst = ps_t.tile([128, 128], f32r, tag="t")
                        nc.tensor.transpose(
                            pst,
                            p_t[:, kt * 128 : (kt + 1) * 128],
                            ident_t[:],
                        )
                        pt_sb = ptpool.tile([128, 128], f32r, tag="pt")
                        nc.vector.tensor_copy(out=pt_sb[:], in_=pst[:])
                        for lo, n in ((0, 512), (512, 256)):  # psum-bank aligned
                            nc.tensor.matmul(
                                po[:, lo : lo + n],
                                pt_sb[:],
                                v_t[:, kt, lo : lo + n],
                                start=(kt == 0),
                                stop=(kt == nkt - 1),
                            )
                    osl = oacc_t[:, i * D : (i + 1) * D]
                    if sb == 0:
                        nc.vector.tensor_copy(out=osl, in_=po[:])
                    else:
                        nc.vector.tensor_add(osl, po[:], osl)

                    if terminal:
                        lsum = spool.tile([128, 1], f32, tag="lsum")
                        nc.vector.tensor_reduce(
                            out=lsum[:],
                            in_=lparts_t[:, i * SB : i * SB + sb + 1],
                            axis=AXX,
                            op=Add,
                        )
                        recip = spool.tile([128, 1], f32, tag="recip")
                        nc.vector.reciprocal(out=recip[:], in_=lsum[:])
                        ob = obpool.tile([128, D], f32, tag="ob")
                        nc.scalar.activation(
                            ob[:], osl, Copy, scale=recip[:, 0:1]
                        )
                        nc.sync.dma_start(
                            out=out_d[i * 128 : (i + 1) * 128, :], in_=ob[:]
                        )
            attn_pools.close()
    if hoist:
        _hoist_multi_waits(nc)
    return nc


def _prep_inputs(x, W_q, W_k, W_v):
    """Per-core input maps. Host-side work is layout only (transposes,
    slicing, mask construction)."""

    def chunked(a):  # [768, N] -> [128, EC, N]
        return np.ascontiguousarray(a.reshape(EC, 128, -1).transpose(1, 0, 2))

    wqT = chunked(W_q.T.copy())
    wkT = chunked(W_k.T.copy())
    wvT = chunked(W_v.T.copy())
    identity = np.eye(128, dtype=np.float32)

    r = np.arange(128, dtype=np.float32)
    tri = np.where(r[None, :] <= r[:, None], 0.0, NEG).astype(np.float32)
    full = np.full((128, 128), NEG, dtype=np.float32)
    zero = np.zeros((128, 128), dtype=np.float32)
    masks = [
        np.ascontiguousarray(np.concatenate([tri, full], axis=1)),  # h = 0
        np.ascontiguousarray(np.concatenate([zero, tri], axis=1)),  # h = 1
    ]

    in_maps = []
    qsels = []
    for c in range(N_CORES):
        b, h = c // 2, c % 2
        xT = chunked(np.ascontiguousarray(x[b].T))  # [128, EC, T]
        qsel = np.concatenate(
            [np.arange((2 * i + h) * 128, (2 * i + h + 1) * 128) for i in range(QTILES)]
        )
        qsels.append(qsel)
        in_maps.append(
            {
                "xkvT": xT,
                "xqT": np.ascontiguousarray(xT[:, :, qsel]),
                "wqT": wqT,
                "wkT": wkT,
                "wvT": wvT,
                "maskab": masks[h],
                "ident": identity,
            }
        )
    return in_maps, qsels


def kernel(x, W_q, W_k, W_v, _trace=False):
    from concourse.bass_utils import run_bass_kernel_spmd

    if "nc" not in _CACHE:
        _CACHE["nc"] = _build_program()
    nc = _CACHE["nc"]

    in_maps, qsels = _prep_inputs(
        np.asarray(x, dtype=np.float32),
        np.asarray(W_q, dtype=np.float32),
        np.asarray(W_k, dtype=np.float32),
        np.asarray(W_v, dtype=np.float32),
    )
    res = run_bass_kernel_spmd(nc, in_maps, list(range(N_CORES)), trace=_trace)
    _CACHE["last_results"] = res

    out = np.empty((B, T, D), dtype=np.float32)
    for c in range(N_CORES):
        b = c // 2
        out[b, qsels[c]] = res.results[c]["out"]
    return out


# revision 18
# speedup vs baseline: 1.0468x; 1.0468x over previous
"""Causal self-attention (B=4, T=4096, D=768, single head, fp32) on 8 TRN2
NeuronCores.

Sharding: core <-> (batch b = core//2, parity h = core%2). Each core handles
the 16 query tiles (128 rows) at global tile index g = 2i + h for local
i = 0..15 (parity interleave balances causal work across the pair to ~3%).
Per local q-tile i the kernel computes scores against keys [0, 256*(i+1)):
columns below 256*i are always causally allowed for both parities; the last
256 columns are fixed up with per-core input mask tiles. This keeps the SPMD
program identical on all cores while wasting only ~6% of the causal-skipped
flops.

Compute path (per core):
  - Q^T projected once to SBUF (lhsT = W_q^T tiles, rhs = streamed x^T).
  - kv processed in 8 superblocks of 512 keys: K^T/V projected just-in-time
    from streamed x^T (double-buffered so next superblock's projection
    overlaps this superblock's attention).
  - Attention computes S^T = K.Q^T directly ([kv, q] layout): the exp of it
    (P^T) is exactly the stationary operand the O = P.V matmul needs, so no
    PE transposes or PSUM->SBUF bounce copies are required. Softmax row sums
    l[q] (a partition-axis reduction in this layout) come from a ones-vector
    matmul that shares P^T's weight load. No max-subtraction pass: scores
    are ~N(0,1) so exp cannot overflow. O and l accumulate per q-tile in
    one PSUM tile across the superblock, then merge into an SBUF
    accumulator; q-tiles normalize by 1/l and DMA out as soon as their key
    range completes. All matmuls run fp32r (FP22) at full PE rate.
"""

import os
import sys
from contextlib import ExitStack

import numpy as np

if "/opt/trn_rl_repo" not in sys.path:
    sys.path.insert(0, "/opt/trn_rl_repo")

B, T, D = 4, 4096, 768
N_CORES = 8
QTILES = 16          # local q-tiles per core, 128 rows each
EC = D // 128        # 6 e/d chunks of 128
SB = 8               # kv superblocks
SBW = 512            # superblock width (keys)
NKT = SBW // 128     # kv 128-tiles per superblock
OSTR = D + 4         # oacc slot stride: [O 768 | l 1 | pad 3]
NEG = -1.0e9
SCALE = 1.0 / float(np.sqrt(D))

_CACHE = {}


def _patch_tile_drain():
    """This walrus build accepts only one sync wait per instruction;
    TileContext's tail drain carries one wait per outstanding proc. Split
    them onto individual SP no-ops (SP executes sequentially, so semantics
    are unchanged)."""
    import concourse.mybir as mybir
    import concourse.tile as tile
    from concourse.vector_clock import ScopedClock

    if getattr(tile.TileContext, "_drain_split_patch", False):
        return

    def _split_drain_and_barrier(self, tick_clock, wait_clock):
        nc = self.nc
        carrier = nc.sync.nop(nofuse=True)
        wait_clock.add_sem_waits(
            carrier.ins, ScopedClock({None: tick_clock.global_clock})
        )
        si = carrier.ins.sync_info
        waits = list(si.on_wait) if si is not None else []
        carrier.ins.sync_info = mybir.SyncInfo(on_wait=waits[:1], on_update=[])
        for w in waits[1:]:
            n = nc.sync.nop(nofuse=True)
            n.ins.sync_info = mybir.SyncInfo(on_wait=[w], on_update=[])
        nc.sync.drain()
        nc.all_engine_barrier()
        assert self.sems is not None
        popped = nc._tile_sem_poison_stack.pop()
        assert popped is self._sem_poison
        nc.clear_and_free_semaphores(list(self.sems.allocated().values()))
        nc.all_engine_barrier()

    tile.TileContext._drain_and_barrier = _split_drain_and_barrier
    tile.TileContext._drain_split_patch = True


def _hoist_multi_waits(nc):
    """This walrus build encodes at most ONE sync wait per instruction
    descriptor. Tile's sem assignment can put several waits on one
    instruction; hoist the extras onto same-engine no-ops inserted
    immediately before it — the engine executes them sequentially, so the
    wait semantics are unchanged."""
    import concourse.mybir as mybir

    n = 0
    for fn in nc.m.functions:
        for bb in fn.blocks:
            insts = bb.instructions
            out = []
            for ins in insts:
                si = ins.sync_info
                waits = list(si.on_wait) if si is not None else []
                if len(waits) > 1:
                    for w in waits[:-1]:
                        nop = mybir.InstNoOp(
                            name=f"I-hoistw-{nc.next_id()}",
                            engine=ins.engine,
                            ins=[],
                            outs=[],
                            sync_info=mybir.SyncInfo(on_wait=[w], on_update=[]),
                        )
                        out.append(nop)
                        n += 1
                    ins.sync_info = mybir.SyncInfo(
                        on_wait=[waits[-1]], on_update=list(si.on_update)
                    )
                out.append(ins)
            insts[:] = out
    return n


def _build_program(hoist=True):
    import concourse.bass as bass
    import concourse.mybir as mybir
    import concourse.tile as tile

    _patch_tile_drain()
    f32 = mybir.dt.float32
    f32r = mybir.dt.float32r
    Exp = mybir.ActivationFunctionType.Exp
    Copy = mybir.ActivationFunctionType.Copy

    nc = bass.Bass()
    xkvT = nc.dram_tensor("xkvT", [128, EC, T], f32r, kind="ExternalInput")
    xqT = nc.dram_tensor("xqT", [128, EC, QTILES * 128], f32r, kind="ExternalInput")
    wqT = nc.dram_tensor("wqT", [128, EC, D], f32r, kind="ExternalInput")
    wkT = nc.dram_tensor("wkT", [128, EC, D], f32r, kind="ExternalInput")
    wvT = nc.dram_tensor("wvT", [128, EC, D], f32r, kind="ExternalInput")
    # masks are [kv, q] (transposed) here; maskf is all -1e9
    maska = nc.dram_tensor("maska", [128, 128], f32, kind="ExternalInput")
    maskb = nc.dram_tensor("maskb", [128, 128], f32, kind="ExternalInput")
    maskf = nc.dram_tensor("maskf", [128, 128], f32, kind="ExternalInput")
    out_d = nc.dram_tensor("out", [QTILES * 128, D], f32, kind="ExternalOutput")

    with tile.TileContext(nc) as tc:
        with (
            tc.tile_pool(name="consts", bufs=1) as cpool,
            tc.tile_pool(name="wk", bufs=1) as wkpool,
            tc.tile_pool(name="qt", bufs=1) as qtpool,
            tc.tile_pool(name="oacc", bufs=1) as oapool,
            tc.tile_pool(name="ps_st", bufs=4, space="PSUM") as ps_st,
            tc.tile_pool(name="ps_o", bufs=2, space="PSUM") as ps_o,
        ):
            ma_t = cpool.tile([128, 128], f32, tag="ma")
            nc.sync.dma_start(out=ma_t[:], in_=maska[:])
            mb_t = cpool.tile([128, 128], f32, tag="mb")
            nc.sync.dma_start(out=mb_t[:], in_=maskb[:])
            mf_t = cpool.tile([128, 128], f32, tag="mf")
            nc.sync.dma_start(out=mf_t[:], in_=maskf[:])
            wk_t = wkpool.tile([128, EC, D], f32r, tag="wk")
            nc.sync.dma_start(out=wk_t[:], in_=wkT[:])
            wv_t = wkpool.tile([128, EC, D], f32r, tag="wv")
            nc.sync.dma_start(out=wv_t[:], in_=wvT[:])

            # ---- Q projection: QT[e_part, e_chunk, q] resident in SBUF ----
            # One weight load per (m, j), 4 moving chunks each.
            qt_t = qtpool.tile([128, EC, QTILES * 128], f32r, tag="qt")
            with tc.tile_pool(name="qproj", bufs=1) as qppool:
                wq_t = qppool.tile([128, EC, D], f32r, tag="wq")
                nc.sync.dma_start(out=wq_t[:], in_=wqT[:])
                xq_ts = []
                for qc in range(4):
                    xq_c = qppool.tile([128, EC, 512], f32r, tag=f"xq{qc}")
                    nc.sync.dma_start(
                        out=xq_c[:], in_=xqT[:, :, qc * 512 : (qc + 1) * 512]
                    )
                    xq_ts.append(xq_c)
                for m in range(EC):
                    pss = [ps_st.tile([128, 512], f32, tag="st", name=f"psq{m}_{qc}") for qc in range(4)]
                    for j in range(EC):
                        for qc in range(4):
                            nc.tensor.matmul(
                                pss[qc][:],
                                wq_t[:, j, m * 128 : (m + 1) * 128],
                                xq_ts[qc][:, j, :],
                                start=(j == 0),
                                stop=(j == EC - 1),
                            )
                    for qc in range(4):
                        nc.scalar.copy(
                            out=qt_t[:, m, qc * 512 : (qc + 1) * 512], in_=pss[qc][:]
                        )

            oacc_t = oapool.tile([128, QTILES * OSTR], f32, tag="oacc")

            # ---- kv superblocks ----
            attn_pools = ExitStack()
            xspool = attn_pools.enter_context(tc.tile_pool(name="xs", bufs=1))
            kvpool = attn_pools.enter_context(tc.tile_pool(name="kv", bufs=2))
            ptpool = attn_pools.enter_context(tc.tile_pool(name="pt", bufs=1))
            spool = attn_pools.enter_context(tc.tile_pool(name="small", bufs=2))
            obpool = attn_pools.enter_context(tc.tile_pool(name="ob", bufs=1))
            for sb in range(SB):
                xkv_t = xspool.tile([128, EC, SBW], f32r, tag="xs")
                nc.sync.dma_start(
                    out=xkv_t[:], in_=xkvT[:, :, sb * SBW : (sb + 1) * SBW]
                )
                # K^T for this superblock: [e_part, e_chunk, kv]
                kt_t = kvpool.tile([128, EC, SBW], f32r, tag="kt")
                for m in range(EC):
                    ps = ps_st.tile([128, SBW], f32, tag="st")
                    for j in range(EC):
                        nc.tensor.matmul(
                            ps[:],
                            wk_t[:, j, m * 128 : (m + 1) * 128],
                            xkv_t[:, j, :],
                            start=(j == 0),
                            stop=(j == EC - 1),
                        )
                    nc.scalar.copy(out=kt_t[:, m, :], in_=ps[:])
                # V for this superblock: [kv_part, kv_tile, e+1]; the
                # extra all-ones column makes the O matmul's second slice
                # also produce the softmax denominator l in the same
                # accumulation group.
                v_t = kvpool.tile([128, NKT, D + 2], f32r, tag="v")
                for kt in range(NKT):
                    ps = ps_o.tile([128, 1024], f32, tag="o")
                    for j in range(EC):
                        for lo, n in ((0, 512), (512, 256)):  # psum-bank aligned
                            nc.tensor.matmul(
                                ps[:, lo : lo + n],
                                xkv_t[:, j, kt * 128 : (kt + 1) * 128],
                                wv_t[:, j, lo : lo + n],
                                start=(j == 0),
                                stop=(j == EC - 1),
                            )
                    nc.scalar.copy(out=v_t[:, kt, :D], in_=ps[:, :D])
                    nc.vector.memset(v_t[:, kt, D : D + 2].bitcast(f32), 1.0)

                # ---- attention, in q-groups of up to 512 columns ----
                # active q-tiles: i in [2*sb, 16); groups are 512-aligned
                i_lo = 2 * sb
                g_lo = i_lo // 4
                for g in range(g_lo, 4):
                    ia = max(i_lo, 4 * g)      # first active q-tile in group
                    ib = 4 * g + 4             # end q-tile (exclusive)
                    qc0 = ia * 128             # first active q column
                    gw = (ib - ia) * 128       # group width (256 or 512)

                    # S^T = K.Q^T for the group's q span, per kv-tile
                    stg = [
                        ps_st.tile([128, 512], f32, tag="st", name=f"st{sb}_{g}_{k}")
                        for k in range(NKT)
                    ]
                    for kt in range(NKT):
                        for j in range(EC):
                            nc.tensor.matmul(
                                stg[kt][:, :gw],
                                kt_t[:, j, kt * 128 : (kt + 1) * 128],
                                qt_t[:, j, qc0 : qc0 + gw],
                                start=(j == 0),
                                stop=(j == EC - 1),
                            )
                    # causal fixups for the terminal q-tiles of this sb:
                    # q-tile 2sb terminates at kv-tiles (0,1) of this sb
                    # (mask a,b; tiles 2,3 fully masked); q-tile 2sb+1
                    # terminates at kv-tiles (2,3).
                    for i, kts in ((2 * sb, (ma_t, mb_t, mf_t, mf_t)),
                                   (2 * sb + 1, (None, None, ma_t, mb_t))):
                        if not (ia <= i < ib):
                            continue
                        qo = i * 128 - qc0
                        for kt in range(NKT):
                            m = kts[kt]
                            if m is not None:
                                nc.vector.tensor_add(
                                    stg[kt][:, qo : qo + 128],
                                    stg[kt][:, qo : qo + 128],
                                    m[:],
                                )
                    # P^T = exp(S^T * scale) back to SBUF
                    pt_t = ptpool.tile([128, NKT, 512], f32r, tag="pt")
                    for kt in range(NKT):
                        nc.scalar.activation(
                            pt_t[:, kt, :gw], stg[kt][:, :gw], Exp, scale=SCALE
                        )
                    # O += P.V and l += P.1 per active q-tile
                    for i in range(ia, ib):
                        qo = i * 128 - qc0
                        po = ps_o.tile([128, 1024], f32, tag="o")
                        for kt in range(NKT):
                            lhs = pt_t[:, kt, qo : qo + 128]
                            for lo, n in ((0, 512), (512, 258)):
                                nc.tensor.matmul(
                                    po[:, lo : lo + n],
                                    lhs,
                                    v_t[:, kt, lo : lo + n],
                                    start=(kt == 0),
                                    stop=(kt == NKT - 1),
                                )
                        osl = oacc_t[:, i * OSTR : i * OSTR + D + 1]
                        if sb == 0:
                            nc.vector.tensor_copy(out=osl, in_=po[:, : D + 1])
                        else:
                            nc.vector.tensor_add(osl, po[:, : D + 1], osl)

                        if i in (2 * sb, 2 * sb + 1):  # retire
                            recip = spool.tile([128, 1], f32, tag="recip")
                            nc.vector.reciprocal(
                                out=recip[:],
                                in_=oacc_t[:, i * OSTR + D : i * OSTR + D + 1],
                            )
                            ob = obpool.tile([128, D], f32, tag="ob")
                            nc.scalar.activation(
                                ob[:],
                                oacc_t[:, i * OSTR : i * OSTR + D],
                                Copy,
                                scale=recip[:, 0:1],
                            )
                            nc.sync.dma_start(
                                out=out_d[i * 128 : (i + 1) * 128, :], in_=ob[:]
                            )
            attn_pools.close()
    if hoist:
        _hoist_multi_waits(nc)
    return nc


def _prep_inputs(x, W_q, W_k, W_v):
    """Per-core input maps. Host-side work is layout only (transposes,
    slicing, mask construction)."""

    def chunked(a):  # [768, N] -> [128, EC, N]
        return np.ascontiguousarray(a.reshape(EC, 128, -1).transpose(1, 0, 2))

    wqT = chunked(W_q.T.copy())
    wkT = chunked(W_k.T.copy())
    wvT = chunked(W_v.T.copy())

    r = np.arange(128, dtype=np.float32)
    # [q, c] triangle: allowed iff c <= q; stored transposed ([kv, q])
    tri = np.where(r[None, :] <= r[:, None], 0.0, NEG).astype(np.float32)
    triT = np.ascontiguousarray(tri.T)
    zero = np.zeros((128, 128), dtype=np.float32)
    full = np.full((128, 128), NEG, dtype=np.float32)
    # per-parity (maska, maskb) for the terminal 256 kv columns
    masks_ab = [(triT, full), (zero, triT)]

    in_maps = []
    qsels = []
    for c in range(N_CORES):
        b, h = c // 2, c % 2
        xT = chunked(np.ascontiguousarray(x[b].T))  # [128, EC, T]
        qsel = np.concatenate(
            [np.arange((2 * i + h) * 128, (2 * i + h + 1) * 128) for i in range(QTILES)]
        )
        qsels.append(qsel)
        ma, mb = masks_ab[h]
        in_maps.append(
            {
                "xkvT": xT,
                "xqT": np.ascontiguousarray(xT[:, :, qsel]),
                "wqT": wqT,
                "wkT": wkT,
                "wvT": wvT,
                "maska": ma,
                "maskb": mb,
                "maskf": full,
            }
        )
    return in_maps, qsels


def kernel(x, W_q, W_k, W_v, _trace=False):
    from concourse.bass_utils import run_bass_kernel_spmd

    if "nc" not in _CACHE:
        _CACHE["nc"] = _build_program()
    nc = _CACHE["nc"]

    in_maps, qsels = _prep_inputs(
        np.asarray(x, dtype=np.float32),
        np.asarray(W_q, dtype=np.float32),
        np.asarray(W_k, dtype=np.float32),
        np.asarray(W_v, dtype=np.float32),
    )
    res = run_bass_kernel_spmd(nc, in_maps, list(range(N_CORES)), trace=_trace)
    _CACHE["last_results"] = res

    out = np.empty((B, T, D), dtype=np.float32)
    for c in range(N_CORES):
        b = c // 2
        out[b, qsels[c]] = res.results[c]["out"]
    return out
